# revision 30
# baseline (speedup 1.0000x reference)
"""Trainium2 Bass kernel for multi-head attention (B=8, N=1024, C=1024, H=16).

Sharding: pure data parallel - one batch element per NeuronCore (8 cores),
no collectives. Host pre-transposes/casts weights and activations to bf16;
all matmuls run bf16 with fp32 PSUM accumulation.

v2 design notes (all-uniform PE tiling):
  - Every matmul runs in the (128,128) PE tiling mode. Scores (K=64 per
    head) are zero-padded to K=128 via per-head "kp" tiles whose dead 64
    rows are memset to 0; the zero rows null the other head's q values
    that ride along in the shared q tile. This removes the (64,128) <->
    (128,128) tiling-mode switches that cost ~100-160ns PE drain each
    (~25-30us/kernel in the previous version) and lets every matmul chain
    issue back-to-back at the 512-free streaming rate.
  - Weights stream in block-major layout (one [128, 8*128] SBUF tile per
    128-column o-block, contraction-tile-major in the free dim) so a
    single 256KB DMA unlocks a complete 16-matmul o-block build. The
    prologue becomes PE-paced after ~4us instead of waiting on full
    row-major weight tiles.
  - kt-outer / half-inner matmul loops reuse each stationary operand for
    both 512-halves (halves LDWEIGHTS pressure).
  - scores come out transposed S^T[nk, nq] with softmax on partitions;
    exp via ScalarE (scale folded, no max subtraction: |s| <= ~4).
  - PV lhsT is a 2-level strided AP [V_h (64 cols) | ones (64 cols)]
    over a shared trailing ones block, so the PV matmul emits O'^T on
    PSUM partitions 0:64 AND the softmax rowsum REPLICATED on partitions
    64:128 -- a free partition-broadcast (M=128 streams at the same rate
    as M=65). Normalization is then 3 local DVE ops (copy rowsum rows to
    SBUF, reciprocal, multiply straight out of PSUM); the previous
    version's ~6us DRAM-bounce broadcast and its gpsimd DMAs are gone.
  - proj chains are dripped into pair 7's filler slots and parked on
    freed PSUM slots right after the last scores, so the tail keeps the
    PE HAM-warm (a starved PE re-throttles to 1.2 GHz); output DMAs are
    bf16 (host upcasts) to halve the drain.
"""

import sys

import numpy as np

if "/opt/trn_rl_repo" not in sys.path:
    sys.path.insert(0, "/opt/trn_rl_repo")

import ml_dtypes

BF16 = ml_dtypes.bfloat16

C = 1024          # model dim
N = 1024          # sequence length
H = 16            # heads
D = 64            # head dim
B = 8             # batch == number of cores
KT = C // 128     # 8 contraction tiles
NT = N // 128     # 8 sequence tiles
SCALE = float(D) ** -0.5

_CACHE = {}
LAST_RESULTS = None


def _build_graph(nc, tc, bass, mybir):
    from contextlib import ExitStack

    f32 = mybir.dt.float32
    bf16 = mybir.dt.bfloat16
    Exp = mybir.ActivationFunctionType.Exp
    Ident = mybir.ActivationFunctionType.Identity

    xT_d = nc.dram_tensor("xT", [C, N], bf16, kind="ExternalInput").ap()
    wqk_d = nc.dram_tensor("wqkB", [16 * 128, 1024], bf16, kind="ExternalInput").ap()
    wv_d = nc.dram_tensor("wvR", [C, 1024], bf16, kind="ExternalInput").ap()
    wp_d = nc.dram_tensor("wpR", [C, 1024], bf16, kind="ExternalInput").ap()
    out_d = nc.dram_tensor("out", [N, C], bf16, kind="ExternalOutput").ap()

    with ExitStack() as ctx:
        persist = ctx.enter_context(tc.tile_pool(name="persist", bufs=1))
        # q/k weight blocks rotate through 6 slots: block p+6's DMA waits
        # (via tile reuse) until build p's last matmul has consumed slot p,
        # which also flow-controls the weight stream behind the builds.
        wqkp = ctx.enter_context(tc.tile_pool(name="wqkp", bufs=6))
        expp = ctx.enter_context(tc.tile_pool(name="expp", bufs=12))
        small = ctx.enter_context(tc.tile_pool(name="small", bufs=2))
        outp = ctx.enter_context(tc.tile_pool(name="outp", bufs=2))
        # PSUM budget = 8 banks: pmm 2x[128,512] (2) + pss 2x[128,1024] (4)
        # + po 2x[128,512] (2).
        pmm = ctx.enter_context(tc.tile_pool(name="pmm", bufs=2, space="PSUM"))
        pss = ctx.enter_context(tc.tile_pool(name="pss", bufs=2, space="PSUM"))
        po = ctx.enter_context(tc.tile_pool(name="po", bufs=2, space="PSUM"))

        # ---- persistent SBUF tensors ----
        xt = [persist.tile([128, N], bf16, tag=f"xt{i}", name=f"xt{i}") for i in range(KT)]
        wv = [persist.tile([128, 1024], bf16, tag=f"wv{i}", name=f"wv{i}") for i in range(KT)]
        wp = [persist.tile([128, 1024], bf16, tag=f"wp{i}", name=f"wp{i}") for i in range(KT)]
        qt = [persist.tile([128, N], bf16, tag=f"qt{i}", name=f"qt{i}") for i in range(NT)]
        kp = [persist.tile([128, N], bf16, tag=f"kp{h}", name=f"kp{h}") for h in range(H)]
        # v tiles: interleaved [v_h (64 cols) | ones (64 cols)] per head so
        # the PV lhsT [128,128] slice emits O'^T on PSUM partitions 0:64
        # and the rowsum replicated on 64:128.
        vv = [persist.tile([128, H * 128], bf16, tag=f"vv{i}", name=f"vv{i}")
              for i in range(NT)]
        ot = [persist.tile([128, N], bf16, tag=f"ot{i}", name=f"ot{i}") for i in range(KT)]

        # ---- input DMAs across THREE queues (sync + scalar HWDGE, gpsimd
        # SWDGE) so the pair-0 critical set (x, q0/k0 blocks, v rows)
        # arrives at full HBM bandwidth. The scalar queue only carries
        # early transfers -- its issuance slots are free before the first
        # exp. gpsimd stays DMA-only so wv/wp issuance isn't stuck behind
        # engine work.
        def xt_dma(eng, i):
            eng.dma_start(xt[i][:], xT_d[i * 128:(i + 1) * 128, :])

        wqk = [None] * 16

        def wqk_dma(eng, b):
            t = wqkp.tile([128, 1024], bf16, tag="wqk", name=f"wqk{b}")
            eng.dma_start(t[:], wqk_d[b * 128:(b + 1) * 128, :])
            wqk[b] = t

        def wv_dma(eng, i):
            eng.dma_start(wv[i][:], wv_d[i * 128:(i + 1) * 128, :])

        xt_dma(nc.sync, 0)
        xt_dma(nc.scalar, 1)
        xt_dma(nc.gpsimd, 2)
        wqk_dma(nc.sync, 0)       # q0 right behind xt0 so builds start ~10us
        wqk_dma(nc.scalar, 8)     # k0
        xt_dma(nc.sync, 3)
        xt_dma(nc.scalar, 4)
        xt_dma(nc.gpsimd, 5)
        xt_dma(nc.sync, 6)
        xt_dma(nc.scalar, 7)
        wqk_dma(nc.sync, 1)       # q1
        wqk_dma(nc.scalar, 9)     # k1
        # wv streams in o-column halves: half 0 (heads 0..7) unblocks the
        # v half-0 builds -- and with them pair-0's PV -- a full 7us
        # earlier than whole-row transfers would.
        for half in range(2):
            for i in range(KT):
                nc.gpsimd.dma_start(wv[i][:, half * 512:(half + 1) * 512],
                                    wv_d[i * 128:(i + 1) * 128,
                                         half * 512:(half + 1) * 512])
        for p in range(2, 8):
            wqk_dma(nc.sync, p)
            wqk_dma(nc.sync, 8 + p)
        for i in range(KT):
            nc.gpsimd.dma_start(wp[i][:], wp_d[i * 128:(i + 1) * 128, :])
        # zero the dead halves of the padded k tiles + set the vv ones
        # blocks on the (otherwise idle) DVE while the DMAs stream; pair-0
        # prerequisites (kp0/kp1, vv ones) come first.
        nc.vector.memset(kp[0][64:128, :], 0.0)
        nc.vector.memset(kp[1][0:64, :], 0.0)
        for i in range(NT):
            ones_view = vv[i][:].rearrange("p (h w) -> p h w", w=128)
            nc.vector.memset(ones_view[:, :, 64:128], 1.0)
        for h in range(2, H):
            if h % 2 == 0:
                nc.vector.memset(kp[h][64:128, :], 0.0)
            else:
                nc.vector.memset(kp[h][0:64, :], 0.0)

        # preload the Exp activation table during the DMA phase so the first
        # real exp doesn't stall the score pipeline ~2.7us.
        warm = small.tile([1, 16], f32, tag="warm", name="warm")
        nc.gpsimd.memset(warm[:], 0.0)
        nc.scalar.activation(warm[:], warm[:], Exp, scale=1.0)

        # ---- builders (incremental so steps can drip as PE filler).
        # All builders accumulate into two [128,512] half tiles from the
        # given pool; the pool choice keeps them off the scores ping-pong.

        def q_builder(p, pool, tag):
            """qt[p][o, n] = wqk[p].T @ xT ; o-block p on partitions."""
            ph = [pool.tile([128, 512], f32, tag=tag, name=f"ps_q{p}_{x}")
                  for x in range(2)]

            def step(kt):
                lhsT = wqk[p][:, kt * 128:(kt + 1) * 128]
                for half in range(2):
                    nc.tensor.matmul(ph[half][:], lhsT,
                                     xt[kt][:, bass.ts(half, 512)],
                                     start=(kt == 0), stop=(kt == KT - 1))

            def finish():
                for half in range(2):
                    nc.vector.tensor_copy(qt[p][:, bass.ts(half, 512)], ph[half][:])

            return step, finish

        def k_builder(p, pool, tag):
            """k o-block p -> padded per-head tiles kp[2p] (rows 0:64 live)
            and kp[2p+1] (rows 64:128 live); dead halves stay memset-0.
            Emitted half-SEQUENTIALLY as an entry list: half 0's matmuls
            and casts complete mid-drip (the next pair's first scores only
            need half 0), half 1 trails with ~4 score-tiles of slack."""
            ph = [pool.tile([128, 512], f32, tag=tag, name=f"ps_k{p}_{x}")
                  for x in range(2)]

            def step(arg):
                kt, half = arg
                lhsT = wqk[8 + p][:, kt * 128:(kt + 1) * 128]
                nc.tensor.matmul(ph[half][:], lhsT,
                                 xt[kt][:, bass.ts(half, 512)],
                                 start=(kt == 0), stop=(kt == KT - 1))

            def finish(half):
                sl = bass.ts(half, 512)
                nc.vector.tensor_copy(kp[2 * p][0:64, sl], ph[half][0:64, :])
                nc.vector.tensor_copy(kp[2 * p + 1][64:128, sl], ph[half][64:128, :])

            ents = []
            for half in range(2):
                ents += [(step, (kt, half)) for kt in range(KT)]
                ents.append((finish, half))
            return ents

        def v_builder(nt, half):
            """One o-half of v[n_tile, o] = xT[:, n].T @ wv (n on
            partitions); half 0 covers heads 0..7 which is all pair 0-3's
            PV needs, so it can run as soon as wv's first half lands."""
            ph = pmm.tile([128, 512], f32, tag="mm", name=f"ps_v{nt}_{half}")

            def step(kt):
                nc.tensor.matmul(ph[:], xt[kt][:, nt * 128:(nt + 1) * 128],
                                 wv[kt][:, bass.ts(half, 512)],
                                 start=(kt == 0), stop=(kt == KT - 1))

            def finish():
                dst = vv[nt][:].rearrange("p (h w) -> p h w", w=128)
                nc.vector.tensor_copy(
                    dst[:, half * 8:(half + 1) * 8, 0:64],
                    ph[:].rearrange("p (h w) -> p h w", w=64))

            return step, finish

        def scores_head(h, j):
            """S^T[nk tile j, nq] for head h: 2 matmuls (shared stationary),
            K padded to 128 by kp's zero rows, then exp -> bf16 SBUF."""
            ps = pss.tile([128, N], f32, tag="s", name=f"ps_s{h}_{j}")
            lhsT = kp[h][:, j * 128:(j + 1) * 128]
            for half in range(2):
                sl = bass.ts(half, 512)
                nc.tensor.matmul(ps[:, sl], lhsT, qt[h // 2][:, sl],
                                 start=True, stop=True)
            e = expp.tile([128, N], bf16, tag="es", name=f"es{h}_{j}")
            nc.scalar.activation(e[:], ps[:], Exp, scale=SCALE)
            return e

        def po_tiles(h):
            return [po.tile([128, 512], f32, tag="o", name=f"pso{h}_{x}")
                    for x in range(2)]

        def pv_step(h, psos, j, e):
            """One nk-tile of accumulation (both nq halves). The lhsT slice
            is [V_h (64 cols) | ones (64 cols)], so the out tile gets O'^T
            on partitions 0:64 and the rowsum replicated on partitions
            64:128 (a free partition-broadcast by the PE; M=128 streams at
            the same rate as M=64)."""
            lhsT = vv[j][:, h * 128:(h + 1) * 128]
            for half in range(2):
                nc.tensor.matmul(psos[half][:], lhsT,
                                 e[:, bass.ts(half, 512)],
                                 start=(j == 0), stop=(j == NT - 1))

        def norm(h, psos):
            """Normalize O'^T by its rowsum into ot: copy the replicated
            rowsum rows to SBUF (the custom-DVE reciprocal misreads PSUM on
            HW), reciprocal, then multiply straight out of PSUM."""
            off = (h % 2) * 64
            for half in range(2):
                sl = bass.ts(half, 512)
                pso = psos[half]
                rs = small.tile([128, 512], f32, tag="rs", name=f"rs{h}_{half}")
                nc.vector.tensor_copy(rs[0:64, :], pso[64:128, :])
                rc = small.tile([128, 512], f32, tag="rc", name=f"rc{h}_{half}")
                nc.vector.reciprocal_approx_fast(out=rc[0:64, :], in_=rs[0:64, :])
                nc.vector.tensor_mul(ot[h // 2][off:off + 64, sl], pso[0:64, :],
                                     rc[0:64, :])

        def proj_builder(nt, pool, tag, full=False, split_finish=False):
            """proj output n-tile: out[n, co] accumulated over kt; each kt
            only needs ot[kt] (heads 2kt, 2kt+1), so kt-steps can drip as
            soon as pair kt's norm ran. full=True puts both halves in one
            [128,1024] slot (pss); otherwise two [128,512] half slots."""
            if full:
                pf = pool.tile([128, N], f32, tag=tag, name=f"ps_p{nt}")
                ph = [pf[:, 0:512], pf[:, 512:1024]]
            else:
                ph = [pool.tile([128, 512], f32, tag=tag, name=f"ps_p{nt}_{x}")[:]
                      for x in range(2)]

            def step(kt):
                lhsT = ot[kt][:, nt * 128:(nt + 1) * 128]
                for half in range(2):
                    nc.tensor.matmul(ph[half], lhsT,
                                     wp[kt][:, bass.ts(half, 512)],
                                     start=(kt == 0), stop=(kt == KT - 1))

            def finish():
                # casts split ACT/DVE (Identity shares the exp table set;
                # the scalar engine is idle once the last exp has drained,
                # which is before any proj finish can run).
                osb = outp.tile([128, N], bf16, tag="osb", name=f"osb{nt}")
                nc.scalar.activation(osb[:, 0:512], ph[0], Ident, scale=1.0)
                nc.vector.tensor_copy(osb[:, 512:1024], ph[1])
                nc.sync.dma_start(out_d[nt * 128:(nt + 1) * 128, :], osb[:])

            if split_finish:
                osb = outp.tile([128, N], bf16, tag="osb", name=f"osb{nt}")
                rsl = slice(nt * 128, (nt + 1) * 128)

                def step_sf(kt):
                    step(kt)
                    if kt == KT - 1:
                        nc.scalar.activation(osb[:, 0:512], ph[0], Ident,
                                             scale=1.0)
                        nc.vector.tensor_copy(osb[:, 512:1024], ph[1])
                        nc.sync.dma_start(out_d[rsl, 0:512], osb[:, 0:512])
                        nc.scalar.dma_start(out_d[rsl, 512:1024],
                                            osb[:, 512:1024])

                return step_sf, None

            return step, finish

        # ---- stage 1 prologue: q0 + k-block 0 paced by the weight DMAs
        # (on the pss slots; scores take them over afterwards).
        q0s, q0f = q_builder(0, pss, "s")
        for kt in range(KT):
            q0s(kt)
        q0f()
        for fn, arg in k_builder(0, pss, "s"):
            fn(arg)

        # Pair-p prerequisites emitted just-in-time: the kp dead-half
        # zeroes and vv ones blocks together cost ~37us of strict-FIFO DVE
        # time if emitted upfront, stalling every builder cast behind
        # them and starving the PE through the whole prologue.
        def prep_pair(p):
            if p > 0:
                nc.vector.memset(kp[2 * p][64:128, :], 0.0)
                nc.vector.memset(kp[2 * p + 1][0:64, :], 0.0)
            for j in range(NT):
                v = vv[j][:].rearrange("p (h w) -> p h w", w=128)
                nc.vector.memset(v[:, 2 * p:2 * p + 2, 64:128], 1.0)

        prep_pair(0)
        prep_pair(1)

        # ---- stage 2, pairs 0..7, software-pipelined per nk-tile j:
        # filler matmuls | scores(h0, j), scores(h1, j) | PV(h0, j-1).
        # Fillers run first in each slot so the PE has ready work while the
        # previous pair's last exp drains the pss slot. Pair 0's fillers
        # are the v builds + pair-1's q/k builds (PV(0,j) consumes vv[j]
        # just after the dripped v_j build completes); pairs 1..6 build
        # pair p+1's q/k tiles; pair 7 drips proj chains nt=0 (pmm) and
        # nt=1 (freed pss slot) -- kt 0..6 only need heads 0..13. filler2
        # builders are constructed AFTER the scores loop so their PSUM
        # allocation does not steal a pss slot from the scores ping-pong.
        def run_filler(lst, fi, n):
            for _ in range(max(0, n)):
                if fi < len(lst):
                    fn, arg = lst[fi]
                    fn(arg) if arg is not None else fn()
                    fi += 1
            return fi

        proj_fin = []
        for pair in range(8):
            h0, h1 = 2 * pair, 2 * pair + 1
            if pair == 0:
                # order: v half-0 (arrives first), then q1/k1 (blocks land
                # ~13us), then v half-1 (arrives ~25us) -- so no FIFO slot
                # ever holds a transfer-gated matmul ahead of ready work.
                filler = []
                for nt in range(NT):
                    vs, vf = v_builder(nt, 0)
                    filler += [(vs, kt) for kt in range(KT)] + [(vf, None)]
                q1s, q1f = q_builder(1, pmm, "mm")
                filler += [(q1s, kt) for kt in range(KT)] + [(q1f, None)]
                filler += k_builder(1, pmm, "mm")
                for nt in range(NT):
                    vs, vf = v_builder(nt, 1)
                    filler += [(vs, kt) for kt in range(KT)] + [(vf, None)]
                take_n = 25
            elif pair < 7:
                qs_, qf_ = q_builder(pair + 1, pmm, "mm")
                filler = [(qs_, kt) for kt in range(KT)] + [(qf_, None)]
                take_n = 1
            else:
                p0s, p0f = proj_builder(0, pmm, "mm")
                proj_fin.append((p0s, p0f))
                filler = [(p0s, kt) for kt in range(KT - 1)]
                take_n = 1
            es = []
            psos0 = po_tiles(h0)
            fi = 0
            for j in range(NT):
                # pair 0's fillers are DMA-gated (v builds): scores go
                # first there so the exp pipeline starts at ~14.5us; in
                # later pairs the fillers lead so the PE has ready work
                # while the previous pair's last exp drains the pss slot.
                if pair == 0:
                    es.append((scores_head(h0, j), scores_head(h1, j)))
                    fi = run_filler(filler, fi, take_n if j < NT - 1 else len(filler) - fi)
                else:
                    fi = run_filler(filler, fi, take_n if j < NT - 1 else len(filler) - fi)
                    es.append((scores_head(h0, j), scores_head(h1, j)))
                if j >= 1:
                    pv_step(h0, psos0, j - 1, es[j - 1][0])
            fi = run_filler(filler, fi, len(filler) - fi)
            if pair < 6:
                prep_pair(pair + 2)
            pv_step(h0, psos0, NT - 1, es[NT - 1][0])
            norm(h0, psos0)
            # filler2: constructed only now (see above). take=4 so the
            # k-build finish copies land early in the PV loop, before the
            # norm DVE chain contends for the vector engine -- otherwise
            # the next pair's scores wait on a late kp cast and the PE
            # starves into a HAM downclock at every pair boundary.
            if pair == 0:
                filler2 = []
                take2_n = 2
            elif pair < 7:
                filler2 = k_builder(pair + 1, pmm, "mm")
                take2_n = 2
            else:
                p1s, p1f = proj_builder(1, pss, "s", full=True)
                proj_fin.append((p1s, p1f))
                filler2 = [(p1s, kt) for kt in range(KT - 1)]
                take2_n = 1
            fi = 0
            psos1 = po_tiles(h1)
            for j in range(NT):
                fi = run_filler(filler2, fi, take2_n if j < NT - 1 else len(filler2) - fi)
                pv_step(h1, psos1, j, es[j][1])
            fi = run_filler(filler2, fi, len(filler2) - fi)
            norm(h1, psos1)

        # ---- stage 3: proj. nt=0 (pmm) and nt=1 (pss) were prebuilt
        # through kt=6 in pair-7's filler slots; park nt=2 on the other pss
        # slot, then finish all three (kt=7 needs norm(14/15)) and run the
        # remaining chains so the PE never idles (no HAM downclock).
        p2s, p2f = proj_builder(2, pss, "s", full=True)
        for kt in range(KT - 1):
            p2s(kt)
        proj_fin.append((p2s, p2f))
        for ps_step, ps_fin in proj_fin:
            ps_step(KT - 1)
            ps_fin()
        # Remaining chains interleaved across three PSUM slot groups so
        # each chain's finish casts overlap the other chains' matmuls.
        ga = [proj_builder(3, po, "o", False), proj_builder(4, pss, "s", True),
              proj_builder(5, pmm, "mm", False)]
        for kt in range(KT):
            for ps_step, _ in ga:
                ps_step(kt)
        for _, ps_fin in ga:
            ps_fin()
        # Final two chains staggered (nt6's casts overlap nt7's matmuls);
        # nt7's finish is split across ACT/DVE and both output DMA queues
        # so the post-last-matmul tail is ~2 engine-ops, not 4 serial ones.
        p6s, p6f = proj_builder(6, pss, "s", True)
        p7s, _ = proj_builder(7, po, "o", False, split_finish=True)
        for kt in range(KT):
            p6s(kt)
        p6f()
        for kt in range(KT):
            p7s(kt)


def _get_compiled():
    key = "nc"
    if key in _CACHE:
        return _CACHE[key]
    import concourse.bass as bass
    import concourse.mybir as mybir
    from concourse import bacc, tile

    nc = bacc.Bacc("TRN2", target_bir_lowering=False, debug=False, num_devices=B)
    with tile.TileContext(nc) as tc:
        _build_graph(nc, tc, bass, mybir)
    nc.compile()
    _CACHE[key] = nc
    return nc


def _in_maps(x, w_qkv, b_qkv, w_proj, b_proj):
    # x^T per batch: [C, N]
    xT = np.ascontiguousarray(np.transpose(np.asarray(x, np.float32), (0, 2, 1))).astype(BF16)
    wT = np.asarray(w_qkv, np.float32).T  # [C, 3C] laid out (c, o)
    # q/k o-blocks, block-major: wqkB[j*128+p, kt*128+cc] = wT[kt*128+p,
    # j*128+cc] so lhsT slices [:, kt*128:(kt+1)*128] are the [c, o] chunks.
    qk = wT[:, :2 * C].reshape(KT, 128, 16, 128)          # [kt, p, j, cc]
    qk = np.ascontiguousarray(qk.transpose(2, 1, 0, 3))   # [j, p, kt, cc]
    wqkB = qk.reshape(16 * 128, 1024).astype(BF16)
    wvR = np.ascontiguousarray(wT[:, 2 * C:]).astype(BF16)       # [C, C]
    wpR = np.ascontiguousarray(np.asarray(w_proj, np.float32).T).astype(BF16)
    return [
        {"xT": np.ascontiguousarray(xT[b]),
         "wqkB": wqkB, "wvR": wvR, "wpR": wpR}
        for b in range(B)
    ]


def _ensure_ntff_hook():
    """The agent image's `antenv` lacks `axon_hooks`, so trace=True would
    crash on import. Provide the registry module and install the ctypes
    hook so neuron-profile NTFF capture works. Only used when tracing."""
    import importlib
    import types

    try:
        importlib.import_module("antenv.axon_hooks")
        return
    except ImportError:
        pass
    mod = types.ModuleType("antenv.axon_hooks")
    mod._hook = None

    def set_axon_ntff_profile_hook(h):
        mod._hook = h

    def get_axon_ntff_profile_hook():
        return mod._hook

    mod.set_axon_ntff_profile_hook = set_axon_ntff_profile_hook
    mod.get_axon_ntff_profile_hook = get_axon_ntff_profile_hook
    import antenv

    antenv.axon_hooks = mod
    sys.modules["antenv.axon_hooks"] = mod
    try:
        from trn_agent_boot.trn_boot import _ntff_profile_via_ctypes

        hook = _ntff_profile_via_ctypes("/opt/axon/libaxon_pjrt.so")
        if hook is not None:
            mod._hook = hook
    except Exception:
        pass


def kernel(x, w_qkv, b_qkv, w_proj, b_proj):
    global LAST_RESULTS
    import os

    if os.environ.get("BASS_TRACE"):
        _ensure_ntff_hook()
    from concourse.bass_utils import run_bass_kernel_spmd

    # biases are zero in this problem; a fold-in path would go here otherwise.
    assert not np.any(np.asarray(b_qkv)) and not np.any(np.asarray(b_proj))
    nc = _get_compiled()
    maps = _in_maps(x, w_qkv, b_qkv, w_proj, b_proj)
    res = run_bass_kernel_spmd(nc, maps, core_ids=list(range(B)))
    LAST_RESULTS = res
    return np.stack([res.results[b]["out"] for b in range(B)]).astype(np.float32)


# revision 32
# speedup vs baseline: 1.0480x; 1.0480x over previous
"""Trainium2 Bass kernel for multi-head attention (B=8, N=1024, C=1024, H=16).

Sharding: pure data parallel - one batch element per NeuronCore (8 cores),
no collectives. Host pre-transposes/casts weights and activations to bf16;
all matmuls run bf16 with fp32 PSUM accumulation.

v2 design notes (all-uniform PE tiling):
  - Every matmul runs in the (128,128) PE tiling mode. Scores (K=64 per
    head) are zero-padded to K=128 via per-head "kp" tiles whose dead 64
    rows are memset to 0; the zero rows null the other head's q values
    that ride along in the shared q tile. This removes the (64,128) <->
    (128,128) tiling-mode switches that cost ~100-160ns PE drain each
    (~25-30us/kernel in the previous version) and lets every matmul chain
    issue back-to-back at the 512-free streaming rate.
  - Weights stream in block-major layout (one [128, 8*128] SBUF tile per
    128-column o-block, contraction-tile-major in the free dim) so a
    single 256KB DMA unlocks a complete 16-matmul o-block build. The
    prologue becomes PE-paced after ~4us instead of waiting on full
    row-major weight tiles.
  - kt-outer / half-inner matmul loops reuse each stationary operand for
    both 512-halves (halves LDWEIGHTS pressure).
  - scores come out transposed S^T[nk, nq] with softmax on partitions;
    exp via ScalarE (scale folded, no max subtraction: |s| <= ~4).
  - PV lhsT is a 2-level strided AP [V_h (64 cols) | ones (64 cols)]
    over a shared trailing ones block, so the PV matmul emits O'^T on
    PSUM partitions 0:64 AND the softmax rowsum REPLICATED on partitions
    64:128 -- a free partition-broadcast (M=128 streams at the same rate
    as M=65). Normalization is then 3 local DVE ops (copy rowsum rows to
    SBUF, reciprocal, multiply straight out of PSUM); the previous
    version's ~6us DRAM-bounce broadcast and its gpsimd DMAs are gone.
  - proj chains are dripped into pair 7's filler slots and parked on
    freed PSUM slots right after the last scores, so the tail keeps the
    PE HAM-warm (a starved PE re-throttles to 1.2 GHz); output DMAs are
    bf16 (host upcasts) to halve the drain.
"""

import sys

import numpy as np

if "/opt/trn_rl_repo" not in sys.path:
    sys.path.insert(0, "/opt/trn_rl_repo")

import ml_dtypes

BF16 = ml_dtypes.bfloat16

C = 1024          # model dim
N = 1024          # sequence length
H = 16            # heads
D = 64            # head dim
B = 8             # batch == number of cores
KT = C // 128     # 8 contraction tiles
NT = N // 128     # 8 sequence tiles
SCALE = float(D) ** -0.5

_CACHE = {}
LAST_RESULTS = None


def _build_graph(nc, tc, bass, mybir):
    from contextlib import ExitStack

    f32 = mybir.dt.float32
    bf16 = mybir.dt.bfloat16
    Exp = mybir.ActivationFunctionType.Exp
    Ident = mybir.ActivationFunctionType.Identity

    xT_d = nc.dram_tensor("xT", [C, N], bf16, kind="ExternalInput").ap()
    wqk_d = nc.dram_tensor("wqkB", [16 * 128, 1024], bf16, kind="ExternalInput").ap()
    wv_d = nc.dram_tensor("wvR", [C, 1024], bf16, kind="ExternalInput").ap()
    wp_d = nc.dram_tensor("wpR", [C, 1024], bf16, kind="ExternalInput").ap()
    out_d = nc.dram_tensor("out", [N, C], bf16, kind="ExternalOutput").ap()

    with ExitStack() as ctx:
        persist = ctx.enter_context(tc.tile_pool(name="persist", bufs=1))
        # q/k weight blocks rotate through 6 slots: block p+6's DMA waits
        # (via tile reuse) until build p's last matmul has consumed slot p,
        # which also flow-controls the weight stream behind the builds.
        wqkp = ctx.enter_context(tc.tile_pool(name="wqkp", bufs=6))
        expp = ctx.enter_context(tc.tile_pool(name="expp", bufs=12))
        small = ctx.enter_context(tc.tile_pool(name="small", bufs=2))
        outp = ctx.enter_context(tc.tile_pool(name="outp", bufs=2))
        # PSUM budget = 8 banks: pmm 2x[128,512] (2) + pss 2x[128,1024] (4)
        # + po 2x[128,512] (2).
        pmm = ctx.enter_context(tc.tile_pool(name="pmm", bufs=2, space="PSUM"))
        pss = ctx.enter_context(tc.tile_pool(name="pss", bufs=2, space="PSUM"))
        po = ctx.enter_context(tc.tile_pool(name="po", bufs=2, space="PSUM"))

        # ---- persistent SBUF tensors ----
        xt = [persist.tile([128, N], bf16, tag=f"xt{i}", name=f"xt{i}") for i in range(KT)]
        wv = [persist.tile([128, 1024], bf16, tag=f"wv{i}", name=f"wv{i}") for i in range(KT)]
        wp = [persist.tile([128, 1024], bf16, tag=f"wp{i}", name=f"wp{i}") for i in range(KT)]
        qt = [persist.tile([128, N], bf16, tag=f"qt{i}", name=f"qt{i}") for i in range(NT)]
        kp = [persist.tile([128, N], bf16, tag=f"kp{h}", name=f"kp{h}") for h in range(H)]
        # v tiles: interleaved [v_h (64 cols) | ones (64 cols)] per head so
        # the PV lhsT [128,128] slice emits O'^T on PSUM partitions 0:64
        # and the rowsum replicated on 64:128.
        vv = [persist.tile([128, H * 128], bf16, tag=f"vv{i}", name=f"vv{i}")
              for i in range(NT)]
        ot = [persist.tile([128, N], bf16, tag=f"ot{i}", name=f"ot{i}") for i in range(KT)]

        # ---- input DMAs across THREE queues (sync + scalar HWDGE, gpsimd
        # SWDGE) so the pair-0 critical set (x, q0/k0 blocks, v rows)
        # arrives at full HBM bandwidth. The scalar queue only carries
        # early transfers -- its issuance slots are free before the first
        # exp. gpsimd stays DMA-only so wv/wp issuance isn't stuck behind
        # engine work.
        def xt_dma(eng, i):
            eng.dma_start(xt[i][:], xT_d[i * 128:(i + 1) * 128, :])

        wqk = [None] * 16

        def wqk_dma(eng, b):
            t = wqkp.tile([128, 1024], bf16, tag="wqk", name=f"wqk{b}")
            eng.dma_start(t[:], wqk_d[b * 128:(b + 1) * 128, :])
            wqk[b] = t

        def wv_dma(eng, i):
            eng.dma_start(wv[i][:], wv_d[i * 128:(i + 1) * 128, :])

        xt_dma(nc.sync, 0)
        xt_dma(nc.scalar, 1)
        xt_dma(nc.gpsimd, 2)
        wqk_dma(nc.sync, 0)       # q0 right behind xt0 so builds start ~10us
        wqk_dma(nc.scalar, 8)     # k0
        xt_dma(nc.sync, 3)
        xt_dma(nc.scalar, 4)
        xt_dma(nc.gpsimd, 5)
        xt_dma(nc.sync, 6)
        xt_dma(nc.scalar, 7)
        wqk_dma(nc.sync, 1)       # q1
        wqk_dma(nc.scalar, 9)     # k1
        # wv streams in o-column halves: half 0 (heads 0..7) unblocks the
        # v half-0 builds -- and with them pair-0's PV -- much earlier
        # than whole-row transfers would. Half 0 rides the scalar HWDGE
        # queue (idle after k1) since the gpsimd SWDGE issuance is slow.
        for i in range(KT):
            nc.scalar.dma_start(wv[i][:, 0:512],
                                wv_d[i * 128:(i + 1) * 128, 0:512])
        for i in range(KT):
            nc.gpsimd.dma_start(wv[i][:, 512:1024],
                                wv_d[i * 128:(i + 1) * 128, 512:1024])
        for p in range(2, 8):
            wqk_dma(nc.sync, p)
            wqk_dma(nc.sync, 8 + p)
        for i in range(KT):
            nc.gpsimd.dma_start(wp[i][:], wp_d[i * 128:(i + 1) * 128, :])
        # zero the dead halves of the padded k tiles + set the vv ones
        # Half masks: the k-builder writes each kp tile as mask x psum (one
        # tensor-scalar multiply per kp per half), so the dead halves are
        # zeroed as a side effect -- no [64,1024] memsets competing with
        # the builder casts in the strict-FIFO DVE queue. The vv ones
        # blocks are emitted just-in-time per pair (prep_pair below).
        msk0 = persist.tile([128, 1], f32, tag="msk0", name="msk0")
        msk1 = persist.tile([128, 1], f32, tag="msk1", name="msk1")
        nc.vector.memset(msk0[0:64, :], 1.0)
        nc.vector.memset(msk0[64:128, :], 0.0)
        nc.vector.memset(msk1[0:64, :], 0.0)
        nc.vector.memset(msk1[64:128, :], 1.0)

        # preload the Exp activation table during the DMA phase so the first
        # real exp doesn't stall the score pipeline ~2.7us.
        warm = small.tile([1, 16], f32, tag="warm", name="warm")
        nc.gpsimd.memset(warm[:], 0.0)
        nc.scalar.activation(warm[:], warm[:], Exp, scale=1.0)

        # ---- builders (incremental so steps can drip as PE filler).
        # All builders accumulate into two [128,512] half tiles from the
        # given pool; the pool choice keeps them off the scores ping-pong.

        def q_builder(p, pool, tag):
            """qt[p][o, n] = wqk[p].T @ xT ; o-block p on partitions."""
            ph = [pool.tile([128, 512], f32, tag=tag, name=f"ps_q{p}_{x}")
                  for x in range(2)]

            def step(kt):
                lhsT = wqk[p][:, kt * 128:(kt + 1) * 128]
                for half in range(2):
                    nc.tensor.matmul(ph[half][:], lhsT,
                                     xt[kt][:, bass.ts(half, 512)],
                                     start=(kt == 0), stop=(kt == KT - 1))

            def finish():
                for half in range(2):
                    nc.vector.tensor_copy(qt[p][:, bass.ts(half, 512)], ph[half][:])

            return step, finish

        def k_builder(p, pool, tag):
            """k o-block p -> padded per-head tiles kp[2p] (rows 0:64 live)
            and kp[2p+1] (rows 64:128 live); dead halves stay memset-0.
            Emitted half-SEQUENTIALLY as an entry list: half 0's matmuls
            and casts complete mid-drip (the next pair's first scores only
            need half 0), half 1 trails with ~4 score-tiles of slack."""
            ph = [pool.tile([128, 512], f32, tag=tag, name=f"ps_k{p}_{x}")
                  for x in range(2)]

            def step(arg):
                kt, half = arg
                lhsT = wqk[8 + p][:, kt * 128:(kt + 1) * 128]
                nc.tensor.matmul(ph[half][:], lhsT,
                                 xt[kt][:, bass.ts(half, 512)],
                                 start=(kt == 0), stop=(kt == KT - 1))

            def finish(half):
                sl = bass.ts(half, 512)
                nc.vector.tensor_scalar_mul(kp[2 * p][:, sl], ph[half][:], msk0[:])
                nc.vector.tensor_scalar_mul(kp[2 * p + 1][:, sl], ph[half][:], msk1[:])

            ents = []
            for half in range(2):
                ents += [(step, (kt, half)) for kt in range(KT)]
                ents.append((finish, half))
            return ents

        def v_builder(nt, half):
            """One o-half of v[n_tile, o] = xT[:, n].T @ wv (n on
            partitions); half 0 covers heads 0..7 which is all pair 0-3's
            PV needs, so it can run as soon as wv's first half lands."""
            ph = pmm.tile([128, 512], f32, tag="mm", name=f"ps_v{nt}_{half}")

            def step(kt):
                nc.tensor.matmul(ph[:], xt[kt][:, nt * 128:(nt + 1) * 128],
                                 wv[kt][:, bass.ts(half, 512)],
                                 start=(kt == 0), stop=(kt == KT - 1))

            def finish():
                dst = vv[nt][:].rearrange("p (h w) -> p h w", w=128)
                nc.vector.tensor_copy(
                    dst[:, half * 8:(half + 1) * 8, 0:64],
                    ph[:].rearrange("p (h w) -> p h w", w=64))

            return step, finish

        def scores_head(h, j):
            """S^T[nk tile j, nq] for head h: 2 matmuls (shared stationary),
            K padded to 128 by kp's zero rows, then exp -> bf16 SBUF."""
            ps = pss.tile([128, N], f32, tag="s", name=f"ps_s{h}_{j}")
            lhsT = kp[h][:, j * 128:(j + 1) * 128]
            for half in range(2):
                sl = bass.ts(half, 512)
                nc.tensor.matmul(ps[:, sl], lhsT, qt[h // 2][:, sl],
                                 start=True, stop=True)
            e = expp.tile([128, N], bf16, tag="es", name=f"es{h}_{j}")
            nc.scalar.activation(e[:], ps[:], Exp, scale=SCALE)
            return e

        def po_tiles(h):
            return [po.tile([128, 512], f32, tag="o", name=f"pso{h}_{x}")
                    for x in range(2)]

        def pv_step(h, psos, j, e):
            """One nk-tile of accumulation (both nq halves). The lhsT slice
            is [V_h (64 cols) | ones (64 cols)], so the out tile gets O'^T
            on partitions 0:64 and the rowsum replicated on partitions
            64:128 (a free partition-broadcast by the PE; M=128 streams at
            the same rate as M=64)."""
            lhsT = vv[j][:, h * 128:(h + 1) * 128]
            for half in range(2):
                nc.tensor.matmul(psos[half][:], lhsT,
                                 e[:, bass.ts(half, 512)],
                                 start=(j == 0), stop=(j == NT - 1))

        def norm(h, psos):
            """Normalize O'^T by its rowsum into ot: copy the replicated
            rowsum rows to SBUF (the custom-DVE reciprocal misreads PSUM on
            HW), reciprocal, then multiply straight out of PSUM."""
            off = (h % 2) * 64
            for half in range(2):
                sl = bass.ts(half, 512)
                pso = psos[half]
                rs = small.tile([128, 512], f32, tag="rs", name=f"rs{h}_{half}")
                nc.vector.tensor_copy(rs[0:64, :], pso[64:128, :])
                rc = small.tile([128, 512], f32, tag="rc", name=f"rc{h}_{half}")
                nc.vector.reciprocal_approx_fast(out=rc[0:64, :], in_=rs[0:64, :])
                nc.vector.tensor_mul(ot[h // 2][off:off + 64, sl], pso[0:64, :],
                                     rc[0:64, :])

        def proj_builder(nt, pool, tag, full=False, split_finish=False):
            """proj output n-tile: out[n, co] accumulated over kt; each kt
            only needs ot[kt] (heads 2kt, 2kt+1), so kt-steps can drip as
            soon as pair kt's norm ran. full=True puts both halves in one
            [128,1024] slot (pss); otherwise two [128,512] half slots."""
            if full:
                pf = pool.tile([128, N], f32, tag=tag, name=f"ps_p{nt}")
                ph = [pf[:, 0:512], pf[:, 512:1024]]
            else:
                ph = [pool.tile([128, 512], f32, tag=tag, name=f"ps_p{nt}_{x}")[:]
                      for x in range(2)]

            def step(kt):
                lhsT = ot[kt][:, nt * 128:(nt + 1) * 128]
                for half in range(2):
                    nc.tensor.matmul(ph[half], lhsT,
                                     wp[kt][:, bass.ts(half, 512)],
                                     start=(kt == 0), stop=(kt == KT - 1))

            def finish():
                # casts split ACT/DVE (Identity shares the exp table set;
                # the scalar engine is idle once the last exp has drained,
                # which is before any proj finish can run).
                osb = outp.tile([128, N], bf16, tag="osb", name=f"osb{nt}")
                nc.scalar.activation(osb[:, 0:512], ph[0], Ident, scale=1.0)
                nc.vector.tensor_copy(osb[:, 512:1024], ph[1])
                nc.sync.dma_start(out_d[nt * 128:(nt + 1) * 128, :], osb[:])

            if split_finish:
                osb = outp.tile([128, N], bf16, tag="osb", name=f"osb{nt}")
                rsl = slice(nt * 128, (nt + 1) * 128)

                def step_sf(kt):
                    step(kt)
                    if kt == KT - 1:
                        nc.scalar.activation(osb[:, 0:512], ph[0], Ident,
                                             scale=1.0)
                        nc.vector.tensor_copy(osb[:, 512:1024], ph[1])
                        nc.sync.dma_start(out_d[rsl, 0:512], osb[:, 0:512])
                        nc.scalar.dma_start(out_d[rsl, 512:1024],
                                            osb[:, 512:1024])

                return step_sf, None

            return step, finish

        # ---- stage 1 prologue: q0 + k-block 0 paced by the weight DMAs
        # (on the pss slots; scores take them over afterwards).
        q0s, q0f = q_builder(0, pss, "s")
        for kt in range(KT):
            q0s(kt)
        q0f()
        for fn, arg in k_builder(0, pss, "s"):
            fn(arg)

        # Pair-p prerequisites emitted just-in-time: the kp dead-half
        # zeroes and vv ones blocks together cost ~37us of strict-FIFO DVE
        # time if emitted upfront, stalling every builder cast behind
        # them and starving the PE through the whole prologue.
        def prep_pair(p):
            for j in range(NT):
                v = vv[j][:].rearrange("p (h w) -> p h w", w=128)
                nc.vector.memset(v[:, 2 * p:2 * p + 2, 64:128], 1.0)

        prep_pair(0)
        prep_pair(1)

        # ---- stage 2, pairs 0..7, software-pipelined per nk-tile j:
        # filler matmuls | scores(h0, j), scores(h1, j) | PV(h0, j-1).
        # Fillers run first in each slot so the PE has ready work while the
        # previous pair's last exp drains the pss slot. Pair 0's fillers
        # are the v builds + pair-1's q/k builds (PV(0,j) consumes vv[j]
        # just after the dripped v_j build completes); pairs 1..6 build
        # pair p+1's q/k tiles; pair 7 drips proj chains nt=0 (pmm) and
        # nt=1 (freed pss slot) -- kt 0..6 only need heads 0..13. filler2
        # builders are constructed AFTER the scores loop so their PSUM
        # allocation does not steal a pss slot from the scores ping-pong.
        def run_filler(lst, fi, n):
            for _ in range(max(0, n)):
                if fi < len(lst):
                    fn, arg = lst[fi]
                    fn(arg) if arg is not None else fn()
                    fi += 1
            return fi

        proj_fin = []
        for pair in range(8):
            h0, h1 = 2 * pair, 2 * pair + 1
            if pair == 0:
                # order: v half-0 (arrives first), then q1/k1 (blocks land
                # ~13us), then v half-1 (arrives ~25us) -- so no FIFO slot
                # ever holds a transfer-gated matmul ahead of ready work.
                q1s, q1f = q_builder(1, pmm, "mm")
                filler = [(q1s, kt) for kt in range(KT)] + [(q1f, None)]
                filler += k_builder(1, pmm, "mm")
                for half in range(2):
                    for nt in range(NT):
                        vs, vf = v_builder(nt, half)
                        filler += [(vs, kt) for kt in range(KT)] + [(vf, None)]
                take_n = 25
            elif pair < 7:
                qs_, qf_ = q_builder(pair + 1, pmm, "mm")
                filler = [(qs_, kt) for kt in range(KT)] + [(qf_, None)]
                take_n = 1
            else:
                p0s, p0f = proj_builder(0, pmm, "mm")
                proj_fin.append((p0s, p0f))
                filler = [(p0s, kt) for kt in range(KT - 1)]
                take_n = 1
            es = []
            psos0 = po_tiles(h0)
            fi = 0
            for j in range(NT):
                # pair 0's fillers are DMA-gated (v builds): scores go
                # first there so the exp pipeline starts at ~14.5us; in
                # later pairs the fillers lead so the PE has ready work
                # while the previous pair's last exp drains the pss slot.
                if pair == 0:
                    es.append((scores_head(h0, j), scores_head(h1, j)))
                    fi = run_filler(filler, fi, take_n if j < NT - 1 else len(filler) - fi)
                else:
                    fi = run_filler(filler, fi, take_n if j < NT - 1 else len(filler) - fi)
                    es.append((scores_head(h0, j), scores_head(h1, j)))
                if j >= 1:
                    pv_step(h0, psos0, j - 1, es[j - 1][0])
            fi = run_filler(filler, fi, len(filler) - fi)
            if pair < 6:
                prep_pair(pair + 2)
            pv_step(h0, psos0, NT - 1, es[NT - 1][0])
            norm(h0, psos0)
            # filler2: constructed only now (see above). take=4 so the
            # k-build finish copies land early in the PV loop, before the
            # norm DVE chain contends for the vector engine -- otherwise
            # the next pair's scores wait on a late kp cast and the PE
            # starves into a HAM downclock at every pair boundary.
            if pair == 0:
                filler2 = []
                take2_n = 2
            elif pair < 7:
                filler2 = k_builder(pair + 1, pmm, "mm")
                take2_n = 2
            else:
                p1s, p1f = proj_builder(1, pss, "s", full=True)
                proj_fin.append((p1s, p1f))
                filler2 = [(p1s, kt) for kt in range(KT - 1)]
                take2_n = 1
            fi = 0
            psos1 = po_tiles(h1)
            for j in range(NT):
                fi = run_filler(filler2, fi, take2_n if j < NT - 1 else len(filler2) - fi)
                pv_step(h1, psos1, j, es[j][1])
            fi = run_filler(filler2, fi, len(filler2) - fi)
            norm(h1, psos1)

        # ---- stage 3: proj. nt=0 (pmm) and nt=1 (pss) were prebuilt
        # through kt=6 in pair-7's filler slots; park nt=2 on the other pss
        # slot, then finish all three (kt=7 needs norm(14/15)) and run the
        # remaining chains so the PE never idles (no HAM downclock).
        p2s, p2f = proj_builder(2, pss, "s", full=True)
        for kt in range(KT - 1):
            p2s(kt)
        proj_fin.append((p2s, p2f))
        for ps_step, ps_fin in proj_fin:
            ps_step(KT - 1)
            ps_fin()
        # Remaining chains interleaved across three PSUM slot groups so
        # each chain's finish casts overlap the other chains' matmuls.
        ga = [proj_builder(3, po, "o", False), proj_builder(4, pss, "s", True),
              proj_builder(5, pmm, "mm", False)]
        for kt in range(KT):
            for ps_step, _ in ga:
                ps_step(kt)
        for _, ps_fin in ga:
            ps_fin()
        # Final two chains staggered (nt6's casts overlap nt7's matmuls);
        # nt7's finish is split across ACT/DVE and both output DMA queues
        # so the post-last-matmul tail is ~2 engine-ops, not 4 serial ones.
        p6s, p6f = proj_builder(6, pss, "s", True)
        p7s, _ = proj_builder(7, po, "o", False, split_finish=True)
        for kt in range(KT):
            p6s(kt)
        p6f()
        for kt in range(KT):
            p7s(kt)


def _get_compiled():
    key = "nc"
    if key in _CACHE:
        return _CACHE[key]
    import concourse.bass as bass
    import concourse.mybir as mybir
    from concourse import bacc, tile

    nc = bacc.Bacc("TRN2", target_bir_lowering=False, debug=False, num_devices=B)
    with tile.TileContext(nc) as tc:
        _build_graph(nc, tc, bass, mybir)
    nc.compile()
    _CACHE[key] = nc
    return nc


def _in_maps(x, w_qkv, b_qkv, w_proj, b_proj):
    # x^T per batch: [C, N]
    xT = np.ascontiguousarray(np.transpose(np.asarray(x, np.float32), (0, 2, 1))).astype(BF16)
    wT = np.asarray(w_qkv, np.float32).T  # [C, 3C] laid out (c, o)
    # q/k o-blocks, block-major: wqkB[j*128+p, kt*128+cc] = wT[kt*128+p,
    # j*128+cc] so lhsT slices [:, kt*128:(kt+1)*128] are the [c, o] chunks.
    qk = wT[:, :2 * C].reshape(KT, 128, 16, 128)          # [kt, p, j, cc]
    qk = np.ascontiguousarray(qk.transpose(2, 1, 0, 3))   # [j, p, kt, cc]
    wqkB = qk.reshape(16 * 128, 1024).astype(BF16)
    wvR = np.ascontiguousarray(wT[:, 2 * C:]).astype(BF16)       # [C, C]
    wpR = np.ascontiguousarray(np.asarray(w_proj, np.float32).T).astype(BF16)
    return [
        {"xT": np.ascontiguousarray(xT[b]),
         "wqkB": wqkB, "wvR": wvR, "wpR": wpR}
        for b in range(B)
    ]


def _ensure_ntff_hook():
    """The agent image's `antenv` lacks `axon_hooks`, so trace=True would
    crash on import. Provide the registry module and install the ctypes
    hook so neuron-profile NTFF capture works. Only used when tracing."""
    import importlib
    import types

    try:
        importlib.import_module("antenv.axon_hooks")
        return
    except ImportError:
        pass
    mod = types.ModuleType("antenv.axon_hooks")
    mod._hook = None

    def set_axon_ntff_profile_hook(h):
        mod._hook = h

    def get_axon_ntff_profile_hook():
        return mod._hook

    mod.set_axon_ntff_profile_hook = set_axon_ntff_profile_hook
    mod.get_axon_ntff_profile_hook = get_axon_ntff_profile_hook
    import antenv

    antenv.axon_hooks = mod
    sys.modules["antenv.axon_hooks"] = mod
    try:
        from trn_agent_boot.trn_boot import _ntff_profile_via_ctypes

        hook = _ntff_profile_via_ctypes("/opt/axon/libaxon_pjrt.so")
        if hook is not None:
            mod._hook = hook
    except Exception:
        pass


def kernel(x, w_qkv, b_qkv, w_proj, b_proj):
    global LAST_RESULTS
    import os

    if os.environ.get("BASS_TRACE"):
        _ensure_ntff_hook()
    from concourse.bass_utils import run_bass_kernel_spmd

    # biases are zero in this problem; a fold-in path would go here otherwise.
    assert not np.any(np.asarray(b_qkv)) and not np.any(np.asarray(b_proj))
    nc = _get_compiled()
    maps = _in_maps(x, w_qkv, b_qkv, w_proj, b_proj)
    res = run_bass_kernel_spmd(nc, maps, core_ids=list(range(B)))
    LAST_RESULTS = res
    return np.stack([res.results[b]["out"] for b in range(B)]).astype(np.float32)
